# revision 11
# baseline (speedup 1.0000x reference)
"""Trainium2 Bass kernel for nn_ControllerLSTM (B=1024, H=2048, NB=12).

Key structural facts exploited:
  * reference() ignores `inputs` and starts h=c=x=0, so every batch row
    computes the IDENTICAL trajectory -> compute one row, broadcast on host.
  * x each step is an embedding row from a tiny table, so x @ W_ih.T + b is
    precomputed once on device (172 rows) and row-selected per step with a
    one-hot matmul driven by the argmax.

Distribution: 8-way tensor parallel over the 4H gate dimension. Core r owns
gate rows for hidden units [r*256,(r+1)*256) of all four gates (order
i,f,o,g). Each step: gates_slice = onehot @ IGATE + h @ WhhT_slice (fp32r
matmuls), sigmoid/tanh on ACT, c/h update on DVE, then a 1KB AllGather of the
new h slice. The hidden contraction axis is tiled as u = 16*p + t (tile t,
partition p) so the post-gather reload is one coarse (128,16) DMA; weight
rows are pre-permuted on the host to match.
"""
import numpy as np

H = 2048
NB = 12
B = 1024
Nb = NB - 1          # 11
NCORE = 8
HS = H // NCORE      # 256 hidden units per core
SL = 4 * HS          # 1024 gate rows per core
KT = 16              # contraction tiles of 128 over H
NSTEPS = 2 * NB - 1  # 23 LSTM cell steps


def _step_table():
    """Per-step metadata.

    For step s: kx/x_off select this step's x row from the igate table using
    the PREVIOUS step's onehot; d_off/kd/kd_mm describe this step's decoder
    (logits) slice of the padded dec matrix.
    """
    # igate (emb) table row offsets; blocks must not cross the 128 row
    # boundary (table is stored as two SBUF tensors of 128 and 44 rows).
    ea = {}
    eb = {}
    off = 1 + 4 * Nb  # A blocks at 1 + 4*(bid-1); B blocks follow
    for bid in range(1, NB):
        ea[bid] = 1 + 4 * (bid - 1)
    for bid in range(1, NB):
        if off < 128 and off + Nb > 128:
            off = 128
        eb[bid] = off
        off += Nb
    n_emb_rows = off  # 172

    # padded dec matrix: 12 act tables of width 4, then 11 block tables
    # padded to width 12 (fp32r needs even moving free dims)
    steps = []
    steps.append(dict(kx=1, x_off=0, d_off=0, kd=4, kd_mm=4))
    for bid in range(1, NB):
        steps.append(dict(kx=4, x_off=ea[bid], d_off=48 + 12 * (bid - 1),
                          kd=bid, kd_mm=12))
        steps.append(dict(kx=bid, x_off=eb[bid], d_off=4 * bid, kd=4, kd_mm=4))
    # selection-count consistency: kx(s) == kd(s-1)
    for s in range(1, len(steps)):
        assert steps[s]["kx"] == steps[s - 1]["kd"]
    return steps, ea, eb, n_emb_rows


STEPS, EA, EB, N_EMB = _step_table()
ND = 48 + 12 * Nb  # 180 padded dec rows


def _build_program(nsteps=NSTEPS, repeats=1):
    import concourse.bacc as bacc
    import concourse.mybir as mybir
    import concourse.tile as tile

    dt = mybir.dt
    nc = bacc.Bacc("TRN2", target_bir_lowering=False, debug=False,
                   enable_asserts=True, num_devices=NCORE)

    # ---- per-core external inputs (host pre-permuted / sliced) ----
    whh = nc.dram_tensor("whh", [H, SL], dt.float32r, kind="ExternalInput").ap()
    wih = nc.dram_tensor("wih", [H, SL], dt.float32r, kind="ExternalInput").ap()
    brow = nc.dram_tensor("brow", [1, SL], dt.float32r, kind="ExternalInput").ap()
    embt = nc.dram_tensor("embt", [H, N_EMB], dt.float32r, kind="ExternalInput").ap()
    embb = nc.dram_tensor("embb", [1, N_EMB], dt.float32r, kind="ExternalInput").ap()
    dect = nc.dram_tensor("dect", [H, ND], dt.float32r, kind="ExternalInput").ap()
    ones1r = nc.dram_tensor("ones1r", [1, 2], dt.float32r, kind="ExternalInput").ap()
    ones1f = nc.dram_tensor("ones1f", [1, 2], dt.float32, kind="ExternalInput").ap()
    iota = nc.dram_tensor("iota", [16, 2], dt.float32r, kind="ExternalInput").ap()

    # ---- outputs ----
    o_h = nc.dram_tensor("o_h", [1, HS], dt.float32, kind="ExternalOutput").ap()
    o_c = nc.dram_tensor("o_c", [1, HS], dt.float32, kind="ExternalOutput").ap()
    o_idx = nc.dram_tensor("o_idx", [1, 2], dt.int32, kind="ExternalOutput").ap()
    o_lg = nc.dram_tensor("o_lg", [1, 16 * nsteps], dt.float32, kind="ExternalOutput").ap()

    AF = mybir.ActivationFunctionType
    OP = mybir.AluOpType

    with tile.TileContext(nc) as tc:
        with (
            tc.tile_pool(name="sb", bufs=1) as sb,
            tc.tile_pool(name="ps", bufs=1, space="PSUM") as ps,
            tc.tile_pool(name="dram", bufs=1, space="DRAM") as dram,
        ):
            # ------- persistent SBUF -------
            whh_sb = sb.tile([128, KT * SL], dt.float32r, tag="whh_sb")
            nc.sync.dma_start(whh_sb[:].rearrange("p (k c) -> p k c", k=KT),
                              whh.rearrange("(k p) c -> p k c", p=128))
            dect_sb = sb.tile([128, KT * ND], dt.float32r, tag="dect_sb")
            nc.sync.dma_start(dect_sb[:].rearrange("p (k c) -> p k c", k=KT),
                              dect.rearrange("(k p) c -> p k c", p=128))
            on1r = sb.tile([1, 2], dt.float32r, tag="on1r")
            nc.sync.dma_start(on1r[:], ones1r[:])
            on1f = sb.tile([1, 2], dt.float32, tag="on1f")
            nc.sync.dma_start(on1f[:], ones1f[:])
            iota_sb = sb.tile([16, 2], dt.float32r, tag="iota_sb")
            nc.sync.dma_start(iota_sb[:], iota[:])
            brow_sb = sb.tile([1, SL], dt.float32r, tag="brow_sb")
            nc.sync.dma_start(brow_sb[:], brow[:])

            c_sb = sb.tile([1, HS], dt.float32, tag="c_sb")
            nc.vector.memset(c_sb[:], 0.0)
            sg = sb.tile([1, 3 * HS], dt.float32, tag="sg")
            tg = sb.tile([1, HS], dt.float32, tag="tg")
            tc2 = sb.tile([1, HS], dt.float32, tag="tc2")
            oh = sb.tile([1, 16], dt.float32, tag="oh")
            mx = sb.tile([1, 2], dt.float32, tag="mx")
            lgtr = sb.tile([1, 16 * nsteps], dt.float32, tag="lgtr")

            # ------- init: igate table = emb_aug @ [W_ih.T; b] -------
            with (
                tc.tile_pool(name="initsb", bufs=1) as isb,
                tc.tile_pool(name="initps", bufs=1, space="PSUM") as ips,
            ):
                embt_sb = isb.tile([128, KT * N_EMB], dt.float32r, tag="embt_sb")
                nc.sync.dma_start(embt_sb[:].rearrange("p (k c) -> p k c", k=KT),
                                  embt.rearrange("(k p) c -> p k c", p=128))
                embb_sb = isb.tile([1, N_EMB], dt.float32r, tag="embb_sb")
                nc.sync.dma_start(embb_sb[:], embb[:])
                p0 = ips.tile([128, SL], mybir.dt.float32, tag="p0")
                p1 = ips.tile([N_EMB - 128, SL], mybir.dt.float32, tag="p1")
                for k in range(KT + 1):
                    if k < KT:
                        wt = isb.tile([128, SL], dt.float32r, name=f"wih{k}",
                                      tag="wihstr", bufs=3)
                        nc.sync.dma_start(wt[:], wih[k * 128:(k + 1) * 128, :])
                        lt = embt_sb[:, k * N_EMB:(k + 1) * N_EMB]
                    else:
                        wt = brow_sb
                        lt = embb_sb
                    for n in range(2):
                        nc.tensor.matmul(
                            p0[:, n * 512:(n + 1) * 512],
                            lt[:, 0:128], wt[:, n * 512:(n + 1) * 512],
                            start=(k == 0), stop=(k == KT))
                        nc.tensor.matmul(
                            p1[:, n * 512:(n + 1) * 512],
                            lt[:, 128:N_EMB], wt[:, n * 512:(n + 1) * 512],
                            start=(k == 0), stop=(k == KT))
                t0full = isb.tile([128, SL], dt.float32r, tag="t0full")
                t1full = isb.tile([N_EMB - 128, SL], dt.float32r, tag="t1full")
                nc.vector.tensor_copy(t0full[:], p0[:])
                nc.vector.tensor_copy(t1full[:], p1[:])
                xgd = dram.tile([N_EMB, SL], dt.float32r, tag="xgd")
                nc.sync.dma_start(xgd[0:128, :], t0full[:])
                nc.sync.dma_start(xgd[128:N_EMB, :], t1full[:])

            # ------- steps -------
            oht_sb = None
            h_r = None
            for rep in range(repeats):
              for s in range(nsteps):
                st = STEPS[s]
                kx, x_off = st["kx"], st["x_off"]

                # --- cell s: gates = h @ WhhT + onehot @ igate ---
                gp = ps.tile([1, SL], mybir.dt.float32, name=f"gp{s}",
                             tag="gates", bufs=1)
                if s > 0 or rep > 0:
                    for k in range(KT):
                        for n in range(2):
                            nc.tensor.matmul(
                                gp[0:1, n * 512:(n + 1) * 512],
                                hT[:, k:k + 1],
                                whh_sb[:, k * SL + n * 512: k * SL + (n + 1) * 512],
                                start=(k == 0), stop=False)
                xgt = sb.tile([16, SL], dt.float32r, name=f"xgt{s}",
                              tag="xgt", bufs=3)
                nc.sync.dma_start(xgt[0:kx, :], xgd[x_off:x_off + kx, :])
                xt = xgt[0:kx, :]
                xl = on1r[0:1, 0:1] if s == 0 else oht_sb[0:kx, 0:1]
                for n in range(2):
                    nc.tensor.matmul(gp[0:1, n * 512:(n + 1) * 512], xl,
                                     xt[:, n * 512:(n + 1) * 512],
                                     start=(s == 0 and rep == 0), stop=True)

                # --- elementwise: i,f,o sigmoid; g tanh; c,h update ---
                nc.scalar.activation(sg[:], gp[0:1, 0:3 * HS], AF.Sigmoid)
                nc.scalar.activation(tg[:], gp[0:1, 3 * HS:SL], AF.Tanh)
                nc.vector.tensor_tensor(tg[:], sg[0:1, 0:HS], tg[:], OP.mult)
                nc.vector.tensor_tensor(c_sb[:], sg[0:1, HS:2 * HS], c_sb[:], OP.mult)
                nc.vector.tensor_tensor(c_sb[:], c_sb[:], tg[:], OP.add)
                nc.scalar.activation(tc2[:], c_sb[:], AF.Tanh)
                h_r = sb.tile([1, HS], dt.float32r, name=f"h_r{s}", tag="h_r", bufs=2)
                nc.vector.tensor_tensor(h_r[:], sg[0:1, 2 * HS:3 * HS], tc2[:], OP.mult)

                # --- AllGather h slice -> full h, mod-16 tiled reload ---
                ag_in = dram.tile([1, HS], dt.float32r, name=f"agi{s}",
                                  tag="ag_in", bufs=2)
                ag_out = dram.tile([128, 16], dt.float32r, name=f"ago{s}",
                                   tag="ag_out", bufs=2)
                nc.sync.dma_start(ag_in[:], h_r[:])
                nc.gpsimd.collective_compute(
                    "AllGather", OP.bypass,
                    replica_groups=[list(range(NCORE))],
                    ins=[ag_in.opt()], outs=[ag_out.opt()])
                hT = sb.tile([128, 16], dt.float32r, name=f"hT{s}", tag="hT", bufs=2)
                nc.sync.dma_start(hT[:], ag_out[:])

                # --- logits for step s decode ---
                d_off, kd, kd_mm = st["d_off"], st["kd"], st["kd_mm"]
                lp = ps.tile([1, 16], mybir.dt.float32, name=f"lp{s}",
                             tag="lg", bufs=1)
                for k in range(KT):
                    nc.tensor.matmul(
                        lp[0:1, 0:kd_mm], hT[:, k:k + 1],
                        dect_sb[:, k * ND + d_off: k * ND + d_off + kd_mm],
                        start=(k == 0), stop=(k == KT - 1))
                nc.vector.tensor_copy(lgtr[0:1, 16 * s:16 * s + kd], lp[0:1, 0:kd])

                # --- argmax -> onehot -> transposed onehot (next step lhsT) ---
                if kd == 1:
                    oht_sb = on1r  # argmax over one candidate: onehot == [1]
                    continue
                nc.vector.tensor_reduce(mx[0:1, 0:1], lp[0:1, 0:kd],
                                        mybir.AxisListType.X, OP.max)
                nc.vector.tensor_scalar(oh[0:1, 0:kd], lp[0:1, 0:kd],
                                        mx[0:1, 0:1], None, OP.is_ge)
                op_t = ps.tile([16, 2], mybir.dt.float32, name=f"opt{s}",
                               tag="oht", bufs=1)
                nc.tensor.transpose(op_t[0:kd, 0:1], oh[0:1, 0:kd], on1f[0:1, 0:1])
                oht_sb = sb.tile([16, 2], dt.float32r, name=f"oht{s}",
                                 tag="oht_sb", bufs=2)
                nc.vector.tensor_copy(oht_sb[0:kd, 0:1], op_t[0:kd, 0:1])

            # ------- final outputs -------
            s = nsteps - 1
            ip = ps.tile([1, 2], mybir.dt.float32, tag="lg", bufs=1)
            nc.tensor.matmul(ip[:], oht_sb[0:STEPS[s]["kd"], 0:1],
                             iota_sb[0:STEPS[s]["kd"], :], start=True, stop=True)
            idx_sb = sb.tile([1, 2], dt.int32, tag="idx_sb")
            nc.vector.tensor_copy(idx_sb[:], ip[:])
            nc.sync.dma_start(o_idx[:], idx_sb[:])

            h_f = sb.tile([1, HS], dt.float32, tag="h_f")
            nc.vector.tensor_tensor(h_f[:], sg[0:1, 2 * HS:3 * HS], tc2[:], OP.mult)
            nc.sync.dma_start(o_h[:], h_f[:])
            nc.sync.dma_start(o_c[:], c_sb[:])
            nc.sync.dma_start(o_lg[:], lgtr[:])

    nc.compile()
    return nc


def _host_shards(inputs):
    """Build per-core input maps (slicing, transposes, K-permutation)."""
    f32 = np.float32
    W_ih = np.ascontiguousarray(inputs["W_ih"], f32)
    W_hh = np.ascontiguousarray(inputs["W_hh"], f32)
    b = np.asarray(inputs["b_ih"], f32) + np.asarray(inputs["b_hh"], f32)
    enc_act = np.asarray(inputs["enc_act"], f32)
    enc_block = np.asarray(inputs["enc_block"], f32)
    dec_act = np.asarray(inputs["dec_act"], f32)
    dec_block = np.asarray(inputs["dec_block"], f32)

    # emb_aug: igate table rows (x embeddings) at the block offsets
    emb = np.zeros((N_EMB, H), f32)
    for bid in range(1, NB):
        emb[EA[bid]:EA[bid] + 4] = enc_act[bid - 1]
        emb[EB[bid]:EB[bid] + Nb] = enc_block[bid - 1]
    embt_full = np.ascontiguousarray(emb.T)          # (H, N_EMB)
    embb = np.ones((1, N_EMB), f32)

    # padded dec matrix (180, H) -> transposed (H, 180)
    dec = np.zeros((ND, H), f32)
    dec[0:48] = dec_act.reshape(48, H)
    for bid in range(1, NB):
        dec[48 + 12 * (bid - 1): 48 + 12 * (bid - 1) + Nb] = dec_block[bid - 1]
    dect_full = np.ascontiguousarray(dec.T)          # (H, ND)

    def kperm(m):  # (H, C) -> row-permuted so tile k = rows [128k:128k+128]
        return np.ascontiguousarray(
            np.concatenate([m[k::KT] for k in range(KT)], axis=0))

    embt_p = kperm(embt_full)
    dect_p = kperm(dect_full)

    ones2 = np.ones((1, 2), f32)
    iota2 = np.repeat(np.arange(16, dtype=f32).reshape(16, 1), 2, axis=1)

    in_maps = []
    for r in range(NCORE):
        rs = r * HS + np.arange(HS)
        sel = np.concatenate([g * H + rs for g in (0, 1, 3, 2)])  # i,f,o,g
        whh_r = kperm(np.ascontiguousarray(W_hh[sel].T))   # (H, SL)
        wih_r = kperm(np.ascontiguousarray(W_ih[sel].T))
        in_maps.append({
            "whh": whh_r, "wih": wih_r,
            "brow": np.ascontiguousarray(b[sel]).reshape(1, SL),
            "embt": embt_p, "embb": embb, "dect": dect_p,
            "ones1r": ones2, "ones1f": ones2, "iota": iota2,
        })
    return in_maps


_CACHE = {}


def _get_program(nsteps=NSTEPS):
    if nsteps not in _CACHE:
        _CACHE[nsteps] = _build_program(nsteps)
    return _CACHE[nsteps]


def run_device(inputs, nsteps=NSTEPS, **run_kwargs):
    """Run the bass kernel; returns (BassKernelResults, in_maps)."""
    from concourse import bass_utils
    nc = _get_program(nsteps)
    in_maps = _host_shards(inputs)
    res = bass_utils.run_bass_kernel_spmd(
        nc, in_maps, core_ids=list(range(NCORE)), **run_kwargs)
    return res


def kernel(**inputs):
    inputs = {k: np.asarray(v) for k, v in inputs.items()}
    res = run_device(inputs)
    h = np.concatenate([res.results[r]["o_h"][0] for r in range(NCORE)])
    c = np.concatenate([res.results[r]["o_c"][0] for r in range(NCORE)])
    idx = np.int32(res.results[0]["o_idx"][0, 0])
    return (
        np.full((B,), idx, np.int32),
        np.ascontiguousarray(np.broadcast_to(h.astype(np.float32), (B, H))),
        np.ascontiguousarray(np.broadcast_to(c.astype(np.float32), (B, H))),
    )
